# revision 42
# baseline (speedup 1.0000x reference)
"""Trainium2 Bass kernel for a 4-step differentiable recurrent net forward pass.

Reference computation (B=8192, NI=512, NH=2048, NO=512, 4 steps):
    activs = 0; outputs = 0
    repeat 4x:  pre = hr * (x @ Wih.T + activs @ Whh.T + outputs @ Woh.T) + hb
                activs = per_neuron_act(pre)        # tanh/sigmoid/relu by i%3
    out = sigmoid(or * (x @ Wio.T + outputs @ Woo.T + activs @ Who.T) + ob)

`outputs` is never written inside the loop, so the Woh/Woo terms vanish and
the x-projection P = hr*(x@Wih.T)+hb is loop-invariant (computed once).

Strategy: data-parallel on batch across 8 cores (1024 rows each). On-core
everything is feature-major (features on SBUF partitions, batch on the free
axis), so each matmul is W_tile.T @ X^T with stationary weights. All matmuls
run in fp8 e4m3 with DoubleRow perf mode (two k-tiles per instruction, 2x
MAC throughput). Weights are scaled by S=256 host-side so their ~0.02-scale
values sit in e4m3's normal range; the 1/S is folded into the activation
instruction's input scale. Activations are quantized to e4m3 unscaled (they
are O(1)). PSUM accumulates in fp32 throughout, so only operand quantization
loses precision (~1.3e-2 rel err on the final sigmoid outputs).

The PE streams fp8-DR matmuls at ~216ns per 512-column instruction (1
col/cycle @2.4GHz), which puts the 920-matmul schedule at a ~198us floor;
everything else is about keeping the PE fed:
  - every weight matrix is host-packed into per-(m-block) pieces that are
    contiguous per partition on BOTH the DRAM and SBUF side and loaded on
    the SP HWDGE queue with max_dma_last_dim=2048, so descriptors are
    1-2KB (the HW path's limit; bigger falls back to the software DGE
    whose arbitration starves the HW queue) and one queue sustains
    ~390GB/s with few triggers;
  - the queue admits ~5 transfers whose descriptors round-robin across
    the 16 DMA engines, so arrival order is only as good as the window:
    the x+wih window flies first and a 4-byte gpsimd gate copy holds the
    1MB whh pieces back until the last x piece has landed;
  - the PE p-state/HAM ramp is absorbed by dummy warmup matmuls gated
    only on a DVE memset, and a dummy Sigmoid ACT up front makes the
    act-table pass pick the one table set covering sigmoid+tanh+relu+
    copy inside the DMA shadow;
  - the P copies out of PSUM (the x-proj phase's real pacer: 18.4us of
    copies vs 13.8us of matmuls) are split between the DVE and scalar
    ACT(Copy), choreographed ahead of the A1 activations, whose own
    deadlines are slack once the hh steps run chunk-sequentially;
  - the output layer accumulates the wio term into the who PSUM groups
    (no outx tile, no DVE adds) and the last tile's ACT/store is split
    384/128 so only a short chain trails the final matmul.
Host-side prep: hidden neurons are permuted so the three activation groups
are contiguous, hr/or are folded into the weight matrices, and hb/ob are
applied as per-partition bias APs inside the ACT instructions.
"""

import os

import numpy as np
import ml_dtypes

import concourse.bass as bass
import concourse.tile as tile
from concourse import bacc, mybir
from concourse.bass_utils import run_bass_kernel_spmd

B, NI, NH, NO = 8192, 512, 2048, 512
N_STEPS = 4
N_CORES = 8
BL = B // N_CORES          # batch rows per core
CH = 512                   # batch chunk (one PSUM bank of fp32)
NCH = BL // CH             # 2 chunks per core
KI = NI // 128             # 4 k-tiles over inputs
KH = NH // 128             # 16 k/m-tiles over hidden
KO = NO // 128             # 4 m-tiles over outputs
NB = KH // 4               # 4 m-blocks of 4 m-tiles over hidden

FP8 = mybir.dt.float8e4
BF16 = mybir.dt.bfloat16
F32 = mybir.dt.float32
AF = mybir.ActivationFunctionType
DR = mybir.MatmulPerfMode.DoubleRow
E4 = ml_dtypes.float8_e4m3

WS = 256.0                 # weight scale into fp8 range
IWS = 1.0 / WS             # folded back out at activation time

# hidden neurons regrouped as [all tanh | all sigmoid | all relu]
_idx = np.arange(NH)
PERM = np.concatenate([_idx[_idx % 3 == 0], _idx[_idx % 3 == 1], _idx[_idx % 3 == 2]])
_B1 = int((_idx % 3 == 0).sum())           # 683
_B2 = _B1 + int((_idx % 3 == 1).sum())     # 1366

# per m-tile: the single activation function, or None for the two mixed tiles
_TILE_FUNC = []
for _m in range(KH):
    _lo, _hi = _m * 128, (_m + 1) * 128
    _fs = set()
    for _f, _a, _b in ((AF.Tanh, 0, _B1), (AF.Sigmoid, _B1, _B2), (AF.Relu, _B2, NH)):
        if max(_lo, _a) < min(_hi, _b):
            _fs.add(_f)
    _TILE_FUNC.append(_fs.pop() if len(_fs) == 1 else None)

# mixed tiles: (major_func applied everywhere, minor_func, mask column block)
# partition sub-ranges must be 32-aligned on TRN2, so the minority strip is
# fixed up with a full-tile ACT + copy_predicated against a {0,1} mask
_BOUNDARY = {
    _B1 // 128: (AF.Sigmoid, AF.Tanh, 0),    # tile 5: parts < 43 are tanh
    _B2 // 128: (AF.Sigmoid, AF.Relu, 1),    # tile 10: parts >= 86 are relu
}


def _emit_hidden_act(nc, ps, blk, a_new, tmp_pool, bmask_t, hbc_t, deferred):
    """Run a 4-m-tile block of WS-scaled pre-activations through the grouped
    activations into a_new, applying the raw hidden bias inside the ACT.

    ps:    AP (128, 4*CH) holding m-tiles blk*4..blk*4+3 side by side
    a_new: SBUF tile (128, KH, CH) fp8, m-tile m lives at [:, m, :]
    hbc_t: (128, KH) f32 per-partition raw biases, column m for m-tile m

    The two mixed tiles' copy_predicated fixups are appended to `deferred`
    instead of emitted inline: the DVE executes in order, and a copy_pred
    stuck behind the scalar-engine ACT backlog would delay the next block's
    PSUM-freeing add (the caller flushes `deferred` after all four adds).
    """
    for mloc in range(4):
        m = blk * 4 + mloc
        bias = hbc_t[:, m:m + 1]
        src = ps[:, mloc * CH:(mloc + 1) * CH]
        if m in _BOUNDARY:
            major, minor, mb = _BOUNDARY[m]
            nc.scalar.activation(a_new[:, m:m + 1, :], src, major,
                                 bias=bias, scale=IWS)
            t = tmp_pool.tile([128, CH], FP8, tag="btmp", bufs=4, name="btmp")
            nc.scalar.activation(t[:], src, minor, bias=bias, scale=IWS)
            deferred.append((a_new[:, m:m + 1, :],
                             bmask_t[:, mb * CH:(mb + 1) * CH], t[:]))
        else:
            nc.scalar.activation(a_new[:, m:m + 1, :], src, _TILE_FUNC[m],
                                 bias=bias, scale=IWS)


def _flush_deferred(nc, deferred):
    for dst, mask, t in deferred:
        nc.vector.copy_predicated(dst, mask, t)
    deferred.clear()


def _build_nc():
    nc = bacc.Bacc("TRN2", target_bir_lowering=False, debug=False,
                   num_devices=N_CORES, dynamic_dma_scratch_size=2048)

    # All operands are host-packed into pieces that are contiguous per
    # partition on both the DRAM and SBUF side, so every DMA descriptor is
    # 1KB+ (the HWDGE descriptor feed caps a queue at ~110GB/s with 512B
    # descriptors, but the 16 shared DMA engines sustain ~350GB/s with
    # 2-8KB ones).
    # Pieces are kept at <=2KB/partition: the HWDGE direct2d path only takes
    # elem_size <= 2048, and larger pieces silently fall back to the software
    # DGE ring, whose engine arbitration starves the HW queue.
    # One trigger per piece with max_dma_last_dim=2048 keeps the descriptors
    # at 2KB while collapsing the trigger count (the SP sequencer spends
    # 565ns per trigger, which was the feed bottleneck with small pieces).
    #   xT:   [128, c*KI+kt, ch]   piece (c) = 2KB/partition
    #   wihp: [mb*128+p, kt, col]  piece (mb) = 2KB/partition
    #   whhp: [mb*128+p, kt, col]  piece (mb) = 8KB/partition
    #   whop: [p, kt, col]         8KB/partition
    xT = nc.dram_tensor("xT", [128, NCH * KI, CH], FP8,
                        kind="ExternalInput").ap()
    wihp = nc.dram_tensor("wihp", [NB * 128, KI, 512], FP8,
                          kind="ExternalInput").ap()
    whhp = nc.dram_tensor("whhp", [NB * 128, KH, 512], FP8,
                          kind="ExternalInput").ap()
    whop = nc.dram_tensor("whop", [128, KH, NO], FP8,
                          kind="ExternalInput").ap()
    wio = nc.dram_tensor("wio", [128, KI, NO], FP8, kind="ExternalInput").ap()
    hbc = nc.dram_tensor("hbc", [128, KH], F32, kind="ExternalInput").ap()
    obc = nc.dram_tensor("obc", [128, KO], F32, kind="ExternalInput").ap()
    bmask = nc.dram_tensor("bmask", [128, 2 * CH], mybir.dt.uint8,
                           kind="ExternalInput").ap()
    outT = nc.dram_tensor("outT", [NO, BL], BF16, kind="ExternalOutput").ap()

    with tile.TileContext(nc) as tc:
        with tc.tile_pool(name="w", bufs=1) as wpool, \
             tc.tile_pool(name="act", bufs=1) as apool, \
             tc.tile_pool(name="ps", bufs=2, space="PSUM") as pspool, \
             tc.tile_pool(name="out", bufs=4) as opool:

            wih_t = [wpool.tile([128, KI, 512], FP8, tag=f"wih{mb}",
                                name=f"wih{mb}") for mb in range(NB)]
            whh_t = [wpool.tile([128, KH, 512], FP8, tag=f"whh{mb}",
                                name=f"whh{mb}") for mb in range(NB)]
            who_m = wpool.tile([128, KH, NO], FP8, tag="who", name="whom")
            x_m = wpool.tile([128, NCH * KI, CH], FP8, tag="x", name="xm")
            wio_m = wpool.tile([128, KI, NO], FP8, tag="wio", name="wiom")
            hbc_t = wpool.tile([128, KH], F32, tag="hbc")
            obc_t = wpool.tile([128, KO], F32, tag="obc")
            bmask_t = wpool.tile([128, 2 * CH], mybir.dt.uint8, tag="bmask")

            # ---- PE warmup: dummy matmuls gated only on a DVE memset (the
            # DVE sequencer comes alive ~1us before gpsimd), so the p-state
            # ramp and HAM clock-gate run against garbage work while the
            # first real operands are still in flight (~10.3us). 12 x 256
            # columns spans ~3us of PE time from a ~7.4us start. ----
            warm_t = wpool.tile([128, 2, 256], FP8, tag="warm", name="warm")
            nc.vector.memset(warm_t[:], 0.0)
            # dummy Sigmoid first so the greedy act-table pass picks the set
            # containing sigmoid+tanh+relu+copy — one table load in the DMA
            # shadow instead of a second 1.28us load mid-ACT-stream
            warm_o = wpool.tile([128, 4], BF16, tag="warmo", name="warmo")
            nc.scalar.activation(warm_o[:], warm_t[:, 0, 0:4], AF.Sigmoid)
            ps_w = pspool.tile([128, 4 * CH], F32, tag="ps", name="psw")
            for _w in range(16):
                nc.tensor.matmul(
                    ps_w[:, (_w % 4) * CH:(_w % 4) * CH + 256],
                    warm_t[:, :, 0:128], warm_t[:],
                    start=True, stop=True, perf_mode=DR,
                    skip_group_check=True)

            # ---- stage all inputs in exact consumption order ----
            # ALL large operands go on the single SP HWDGE queue: with 1-8KB
            # descriptors one queue saturates the 16 shared DMA engines
            # (~360GB/s measured), and strict FIFO makes arrival order equal
            # consumption order. (A parallel gpsimd software-DGE stream for
            # whh was tried and starved the x/wih stream it was supposed to
            # overlap with — the engines favor the software queue.)
            # The queue admits ~5 transfers in flight and the engines
            # round-robin across them, so a window of pieces completes
            # roughly together. The x-proj operands go first and fly solo;
            # the whh stream is held back by a 4-byte DVE "gate" copy that
            # reads the tail of the last x piece and writes a corner of
            # whh_t[0] — the whh trigger's write-after-write dependency on
            # that corner keeps the 1MB whh pieces out of the in-flight
            # window until the x stream has landed.
            # The first-needed x-c0/wih-mb0 pieces are split in two: the
            # in-flight window shares engine bandwidth per transfer, so two
            # half-size transfers complete ~2x sooner than one — the first
            # real matmul starts ~1us earlier.
            MD = dict(max_dma_last_dim=2048)
            nc.sync.dma_start(x_m[:, 0:2, :], xT[:, 0:2, :], **MD)
            nc.sync.dma_start(wih_t[0][:, 0:2, :], wihp[0:128, 0:2, :], **MD)
            nc.sync.dma_start(x_m[:, 2:4, :], xT[:, 2:4, :], **MD)
            nc.sync.dma_start(wih_t[0][:, 2:4, :], wihp[0:128, 2:4, :], **MD)
            for mb in range(1, NB):
                nc.sync.dma_start(wih_t[mb][:],
                                  wihp[mb * 128:(mb + 1) * 128], **MD)
            nc.sync.dma_start(x_m[:, KI:2 * KI, :], xT[:, KI:2 * KI, :], **MD)
            # (the gate runs on gpsimd — it is idle, and at the head of the
            # DVE queue the wait for x-c1 would push the whole cast chain)
            nc.gpsimd.tensor_copy(whh_t[0][0:1, 0:1, 0:4],
                                  x_m[0:1, 2 * KI - 1:2 * KI, CH - 4:CH])
            for mb in range(NB):
                nc.sync.dma_start(whh_t[mb][:],
                                  whhp[mb * 128:(mb + 1) * 128], **MD)
            nc.sync.dma_start(who_m[:], whop[:], **MD)
            # ACT HWDGE queue: only the small early operands (more triggers
            # here would eat scalar-sequencer time the ACT backlog needs).
            nc.scalar.dma_start(hbc_t[:], hbc[:])
            nc.scalar.dma_start(bmask_t[:], bmask[:])
            nc.scalar.dma_start(wio_m[:], wio[:])
            nc.scalar.dma_start(obc_t[:], obc[:])

            # ---- per-chunk x-projection P and first-step activations.
            # P holds the raw WS-scaled x-projection (bias is applied inside
            # the ACTs); the copy out of PSUM frees the slot for the next
            # group, and the hh phase can't start until c1's last two slots
            # are freed. The DVE alone would need 18.4us for all eight
            # copies vs 13.8us of x-proj PE work, so c1's blk1-3 copy via
            # scalar ACT(Copy); emission is choreographed so those casts run
            # ahead of c0's blk3 ACTs and all of c1's ACTs in the scalar
            # queue — the A1 activations they displace have 25+us of
            # deadline slack, the PSUM slots don't. ----
            P = {}
            A = {}
            a1 = {}
            deferred = {0: [], 1: []}
            late_acts = {0: [], 1: []}
            for c in range(NCH):
                P[c] = apool.tile([128, KH * CH], BF16, tag=f"P{c}",
                                  name=f"P{c}")
                a1[c] = apool.tile([128, KH, CH], FP8, tag="A", bufs=3,
                                   name=f"A1c{c}")
                for blk in range(4):
                    ps = pspool.tile([128, 4 * CH], F32, tag="ps", name="psb")
                    for kp in range(KI // 2):
                        for mloc in range(4):
                            nc.tensor.matmul(
                                ps[:, mloc * CH:(mloc + 1) * CH],
                                wih_t[blk][:, 2 * kp:2 * kp + 2,
                                           mloc * 128:(mloc + 1) * 128],
                                x_m[:, c * KI + 2 * kp:c * KI + 2 * kp + 2, :],
                                start=(kp == 0), stop=(kp == KI // 2 - 1),
                                perf_mode=DR)
                    dst = P[c][:, blk * 4 * CH:(blk + 1) * 4 * CH]
                    if c == 1 and blk >= 1:
                        nc.scalar.activation(dst, ps[:], AF.Copy)
                    else:
                        nc.vector.tensor_copy(dst, ps[:])
                    if c == 0 and blk < 3:
                        _emit_hidden_act(nc, dst, blk, a1[c], opool, bmask_t,
                                         hbc_t, deferred[c])
                    else:
                        late_acts[c].append((dst, blk))
            for c in range(NCH):
                for dst, blk in late_acts[c]:
                    _emit_hidden_act(nc, dst, blk, a1[c], opool, bmask_t,
                                     hbc_t, deferred[c])
                _flush_deferred(nc, deferred[c])
                A[c] = a1[c]

            # ---- recurrent steps 2..4, one chunk at a time: chunk c's
            # next-step consumer is a full 27.6us behind its producer, so
            # the scalar ACT backlog (11us per chunk-step) never gates ----
            def hh_step(c, s):
                a_new = apool.tile([128, KH, CH], FP8, tag="A", bufs=3,
                                   name=f"A{s + 2}c{c}")
                deferred = []
                for blk in range(4):
                    ps = pspool.tile([128, 4 * CH], F32, tag="ps", name="psb")
                    for kp in range(KH // 2):
                        for mloc in range(4):
                            nc.tensor.matmul(
                                ps[:, mloc * CH:(mloc + 1) * CH],
                                whh_t[blk][:, 2 * kp:2 * kp + 2,
                                           mloc * 128:(mloc + 1) * 128],
                                A[c][:, 2 * kp:2 * kp + 2, :],
                                start=(kp == 0), stop=(kp == KH // 2 - 1),
                                perf_mode=DR)
                    # pre = psum + P into an SBUF temp: a single PSUM read
                    # frees the bank; ACT then runs off SBUF
                    tmp = opool.tile([128, 4 * CH], F32, tag="pre", bufs=2,
                                     name="pre")
                    nc.vector.tensor_add(
                        tmp[:], ps[:],
                        P[c][:, blk * 4 * CH:(blk + 1) * 4 * CH])
                    _emit_hidden_act(nc, tmp, blk, a_new, opool, bmask_t,
                                     hbc_t, deferred)
                _flush_deferred(nc, deferred)
                A[c] = a_new

            for s in range(N_STEPS - 1):
                for c in range(NCH):
                    hh_step(c, s)

            # ---- output layer; chunk 0's output post-chain overlaps chunk
            # 1's final hh step and output matmuls ----
            # ---- output layer: the x-projection (wio) matmuls open each
            # PSUM accumulation group and the who matmuls close it, so the
            # whole out pre-activation stays in f32 PSUM and the sigmoid ACT
            # reads PSUM directly — no SBUF outx tile, no DVE adds, and the
            # tail chain after the last matmul is just ACT + store ----
            def out_chunk(c):
                last = (c == NCH - 1)
                for mo in range(KO):
                    pso = pspool.tile([128, CH], F32, tag="ps", name="pso")
                    oap = pso[:]
                    for kp in range(KI // 2):
                        nc.tensor.matmul(
                            oap,
                            wio_m[:, 2 * kp:2 * kp + 2,
                                  mo * 128:(mo + 1) * 128],
                            x_m[:, c * KI + 2 * kp:c * KI + 2 * kp + 2, :],
                            start=(kp == 0), stop=False, perf_mode=DR)
                    for kp in range(KH // 2):
                        nc.tensor.matmul(
                            oap,
                            who_m[:, 2 * kp:2 * kp + 2,
                                  mo * 128:(mo + 1) * 128],
                            A[c][:, 2 * kp:2 * kp + 2, :],
                            start=False, stop=(kp == KH // 2 - 1),
                            perf_mode=DR)
                    o = opool.tile([128, CH], BF16, tag="o", bufs=2, name="o")
                    nc.scalar.activation(o[:], oap, AF.Sigmoid,
                                         bias=obc_t[:, mo:mo + 1],
                                         scale=IWS)
                    # the very last store rides the SP queue: its trigger
                    # pre-fetches and fires right on the ACT's semaphore,
                    # and one 512-wide chain beats two half-chains whose
                    # serial 0.6us triggers outweigh the overlap
                    eng = nc.sync if (mo % 2 == 0 or (last and mo == KO - 1)) \
                        else nc.scalar
                    eng.dma_start(
                        outT[mo * 128:(mo + 1) * 128, c * CH:(c + 1) * CH],
                        o[:])

            out_chunk(0)
            out_chunk(1)

    nc.compile()
    return nc


_NC_CACHE = None


def _get_nc():
    global _NC_CACHE
    if _NC_CACHE is None:
        _NC_CACHE = _build_nc()
    return _NC_CACHE


def _make_bmask():
    m = np.zeros((128, 2 * CH), np.uint8)
    m[:_B1 - (_B1 // 128) * 128, 0:CH] = 1          # tile 5: parts < 43 tanh
    m[_B2 - (_B2 // 128) * 128:, CH:2 * CH] = 1     # tile 10: parts >= 86 relu
    return m


def _q8(a):
    return np.clip(a, -240.0, 240.0).astype(E4)


def _prep_in_maps(inputs):
    x = np.asarray(inputs["inputs"], np.float32)
    hr = np.asarray(inputs["hidden_responses"], np.float32)[PERM]
    hb = np.asarray(inputs["hidden_biases"], np.float32)[PERM]
    orr = np.asarray(inputs["output_responses"], np.float32)
    ob = np.asarray(inputs["output_biases"], np.float32)

    wih_s = WS * (hr[:, None] * np.asarray(inputs["input_to_hidden"], np.float32)[PERM]).T
    whh_s = WS * (hr[:, None] *
                  np.asarray(inputs["hidden_to_hidden"], np.float32)[PERM][:, PERM]).T
    who_s = WS * (orr[:, None] *
                  np.asarray(inputs["hidden_to_output"], np.float32)[:, PERM]).T
    wio_s = WS * (orr[:, None] * np.asarray(inputs["input_to_output"], np.float32)).T

    def pack(w, ktiles):     # (ktiles*128, C) -> (128, ktiles, C)
        c = w.shape[1]
        return np.ascontiguousarray(
            w.reshape(ktiles, 128, c).transpose(1, 0, 2))

    def pack_mb(w, ktiles, mw, kw=None):
        # (ktiles*128, C) -> pieces [(mb, kq)][128, kw, mw] stacked on dim0:
        # piece (mb, kq) holds k-tiles kq*kw..(kq+1)*kw of columns
        # mb*mw..(mb+1)*mw, contiguous per partition on both sides
        kw = kw or ktiles
        p = pack(w, ktiles)                       # (128, ktiles, C)
        nmb, nkq = p.shape[2] // mw, ktiles // kw
        return np.ascontiguousarray(
            p.reshape(128, nkq, kw, nmb, mw).transpose(3, 1, 0, 2, 4)
            .reshape(nmb * nkq * 128, kw, mw))

    shared = {
        "wihp": _q8(pack_mb(wih_s, KI, 512)),
        "whhp": _q8(pack_mb(whh_s, KH, 512)),
        "whop": _q8(pack(who_s, KH)),
        "wio": _q8(pack(wio_s, KI)),
        "hbc": np.ascontiguousarray(hb.reshape(KH, 128).T),
        "obc": np.ascontiguousarray(ob.reshape(KO, 128).T),
        "bmask": _make_bmask(),
    }
    in_maps = []
    for c in range(N_CORES):
        m = dict(shared)
        xp = pack(np.ascontiguousarray(x[c * BL:(c + 1) * BL].T), KI)
        m["xT"] = _q8(np.ascontiguousarray(
            xp.reshape(128, KI, NCH, CH).transpose(0, 2, 1, 3)
            .reshape(128, NCH * KI, CH)))
        in_maps.append(m)
    return in_maps


def _run(inputs, trace=False, tmpdir=None):
    nc = _get_nc()
    in_maps = _prep_in_maps(inputs)
    res = run_bass_kernel_spmd(nc, in_maps, core_ids=list(range(N_CORES)),
                               trace=trace, tmpdir=tmpdir)
    out = np.empty((B, NO), np.float32)
    for c in range(N_CORES):
        out[c * BL:(c + 1) * BL] = res.results[c]["outT"].T.astype(np.float32)
    return out, res


def kernel(**inputs) -> np.ndarray:
    out, _ = _run(inputs, trace=False)
    return out


if __name__ == "__main__":
    rng = np.random.default_rng(0)
    ins = {
        "inputs": rng.standard_normal((B, NI), dtype=np.float32),
        "input_to_hidden": rng.standard_normal((NH, NI), dtype=np.float32) * 0.02,
        "hidden_to_hidden": rng.standard_normal((NH, NH), dtype=np.float32) * 0.02,
        "output_to_hidden": rng.standard_normal((NH, NO), dtype=np.float32) * 0.02,
        "input_to_output": rng.standard_normal((NO, NI), dtype=np.float32) * 0.02,
        "hidden_to_output": rng.standard_normal((NO, NH), dtype=np.float32) * 0.02,
        "output_to_output": rng.standard_normal((NO, NO), dtype=np.float32) * 0.02,
        "hidden_responses": rng.standard_normal(NH, dtype=np.float32) * 0.1 + 1.0,
        "hidden_biases": rng.standard_normal(NH, dtype=np.float32) * 0.1,
        "output_responses": rng.standard_normal(NO, dtype=np.float32) * 0.1 + 1.0,
        "output_biases": rng.standard_normal(NO, dtype=np.float32) * 0.1,
    }
    out = kernel(**ins)
    print("kernel output", out.shape, out.dtype, out[:2, :4])


# revision 45
# speedup vs baseline: 1.0072x; 1.0072x over previous
"""Trainium2 Bass kernel for a 4-step differentiable recurrent net forward pass.

Reference computation (B=8192, NI=512, NH=2048, NO=512, 4 steps):
    activs = 0; outputs = 0
    repeat 4x:  pre = hr * (x @ Wih.T + activs @ Whh.T + outputs @ Woh.T) + hb
                activs = per_neuron_act(pre)        # tanh/sigmoid/relu by i%3
    out = sigmoid(or * (x @ Wio.T + outputs @ Woo.T + activs @ Who.T) + ob)

`outputs` is never written inside the loop, so the Woh/Woo terms vanish and
the x-projection P = hr*(x@Wih.T)+hb is loop-invariant (computed once).

Strategy: data-parallel on batch across 8 cores (1024 rows each). On-core
everything is feature-major (features on SBUF partitions, batch on the free
axis), so each matmul is W_tile.T @ X^T with stationary weights. All matmuls
run in fp8 e4m3 with DoubleRow perf mode (two k-tiles per instruction, 2x
MAC throughput). Weights are scaled by S=256 host-side so their ~0.02-scale
values sit in e4m3's normal range; the 1/S is folded into the activation
instruction's input scale. Activations are quantized to e4m3 unscaled (they
are O(1)). PSUM accumulates in fp32 throughout, so only operand quantization
loses precision (~1.3e-2 rel err on the final sigmoid outputs).

The PE streams fp8-DR matmuls at ~216ns per 512-column instruction (1
col/cycle @2.4GHz), which puts the 920-matmul schedule at a ~198us floor;
everything else is about keeping the PE fed:
  - every weight matrix is host-packed into per-(m-block) pieces that are
    contiguous per partition on BOTH the DRAM and SBUF side and loaded on
    the SP HWDGE queue with max_dma_last_dim=2048, so descriptors are
    1-2KB (the HW path's limit; bigger falls back to the software DGE
    whose arbitration starves the HW queue) and one queue sustains
    ~390GB/s with few triggers;
  - the queue admits ~5 transfers whose descriptors round-robin across
    the 16 DMA engines, so arrival order is only as good as the window:
    the x+wih window flies first and a 4-byte gpsimd gate copy holds the
    1MB whh pieces back until the last x piece has landed;
  - the PE p-state/HAM ramp is absorbed by dummy warmup matmuls gated
    only on a DVE memset, and a dummy Sigmoid ACT up front makes the
    act-table pass pick the one table set covering sigmoid+tanh+relu+
    copy inside the DMA shadow;
  - the P copies out of PSUM (the x-proj phase's real pacer: 18.4us of
    copies vs 13.8us of matmuls) are split between the DVE and scalar
    ACT(Copy), choreographed ahead of the A1 activations, whose own
    deadlines are slack once the hh steps run chunk-sequentially;
  - the output layer accumulates the wio term into the who PSUM groups
    (no outx tile, no DVE adds) and the last tile's ACT/store is split
    384/128 so only a short chain trails the final matmul.
Host-side prep: hidden neurons are permuted so the three activation groups
are contiguous, hr/or are folded into the weight matrices, and hb/ob are
applied as per-partition bias APs inside the ACT instructions.
"""

import os

import numpy as np
import ml_dtypes

import concourse.bass as bass
import concourse.tile as tile
from concourse import bacc, mybir
from concourse.bass_utils import run_bass_kernel_spmd

B, NI, NH, NO = 8192, 512, 2048, 512
N_STEPS = 4
N_CORES = 8
BL = B // N_CORES          # batch rows per core
CH = 512                   # batch chunk (one PSUM bank of fp32)
NCH = BL // CH             # 2 chunks per core
KI = NI // 128             # 4 k-tiles over inputs
KH = NH // 128             # 16 k/m-tiles over hidden
KO = NO // 128             # 4 m-tiles over outputs
NB = KH // 4               # 4 m-blocks of 4 m-tiles over hidden

FP8 = mybir.dt.float8e4
BF16 = mybir.dt.bfloat16
F32 = mybir.dt.float32
AF = mybir.ActivationFunctionType
DR = mybir.MatmulPerfMode.DoubleRow
E4 = ml_dtypes.float8_e4m3

WS = 256.0                 # weight scale into fp8 range
IWS = 1.0 / WS             # folded back out at activation time

# hidden neurons regrouped as [all tanh | all sigmoid | all relu]
_idx = np.arange(NH)
PERM = np.concatenate([_idx[_idx % 3 == 0], _idx[_idx % 3 == 1], _idx[_idx % 3 == 2]])
_B1 = int((_idx % 3 == 0).sum())           # 683
_B2 = _B1 + int((_idx % 3 == 1).sum())     # 1366

# per m-tile: the single activation function, or None for the two mixed tiles
_TILE_FUNC = []
for _m in range(KH):
    _lo, _hi = _m * 128, (_m + 1) * 128
    _fs = set()
    for _f, _a, _b in ((AF.Tanh, 0, _B1), (AF.Sigmoid, _B1, _B2), (AF.Relu, _B2, NH)):
        if max(_lo, _a) < min(_hi, _b):
            _fs.add(_f)
    _TILE_FUNC.append(_fs.pop() if len(_fs) == 1 else None)

# mixed tiles: (major_func applied everywhere, minor_func, mask column block)
# partition sub-ranges must be 32-aligned on TRN2, so the minority strip is
# fixed up with a full-tile ACT + copy_predicated against a {0,1} mask
_BOUNDARY = {
    _B1 // 128: (AF.Sigmoid, AF.Tanh, 0),    # tile 5: parts < 43 are tanh
    _B2 // 128: (AF.Sigmoid, AF.Relu, 1),    # tile 10: parts >= 86 are relu
}


def _emit_hidden_act(nc, ps, blk, a_new, tmp_pool, bmask_t, hbc_t, deferred):
    """Run a 4-m-tile block of WS-scaled pre-activations through the grouped
    activations into a_new, applying the raw hidden bias inside the ACT.

    ps:    AP (128, 4*CH) holding m-tiles blk*4..blk*4+3 side by side
    a_new: SBUF tile (128, KH, CH) fp8, m-tile m lives at [:, m, :]
    hbc_t: (128, KH) f32 per-partition raw biases, column m for m-tile m

    The two mixed tiles' copy_predicated fixups are appended to `deferred`
    instead of emitted inline: the DVE executes in order, and a copy_pred
    stuck behind the scalar-engine ACT backlog would delay the next block's
    PSUM-freeing add (the caller flushes `deferred` after all four adds).
    """
    for mloc in range(4):
        m = blk * 4 + mloc
        bias = hbc_t[:, m:m + 1]
        src = ps[:, mloc * CH:(mloc + 1) * CH]
        if m in _BOUNDARY:
            major, minor, mb = _BOUNDARY[m]
            nc.scalar.activation(a_new[:, m:m + 1, :], src, major,
                                 bias=bias, scale=IWS)
            t = tmp_pool.tile([128, CH], FP8, tag="btmp", bufs=4, name="btmp")
            nc.scalar.activation(t[:], src, minor, bias=bias, scale=IWS)
            deferred.append((a_new[:, m:m + 1, :],
                             bmask_t[:, mb * CH:(mb + 1) * CH], t[:]))
        else:
            nc.scalar.activation(a_new[:, m:m + 1, :], src, _TILE_FUNC[m],
                                 bias=bias, scale=IWS)


def _flush_deferred(nc, deferred):
    for dst, mask, t in deferred:
        nc.vector.copy_predicated(dst, mask, t)
    deferred.clear()


def _build_nc():
    nc = bacc.Bacc("TRN2", target_bir_lowering=False, debug=False,
                   num_devices=N_CORES, dynamic_dma_scratch_size=2048)

    # All operands are host-packed into pieces that are contiguous per
    # partition on both the DRAM and SBUF side, so every DMA descriptor is
    # 1KB+ (the HWDGE descriptor feed caps a queue at ~110GB/s with 512B
    # descriptors, but the 16 shared DMA engines sustain ~350GB/s with
    # 2-8KB ones).
    # Pieces are kept at <=2KB/partition: the HWDGE direct2d path only takes
    # elem_size <= 2048, and larger pieces silently fall back to the software
    # DGE ring, whose engine arbitration starves the HW queue.
    # One trigger per piece with max_dma_last_dim=2048 keeps the descriptors
    # at 2KB while collapsing the trigger count (the SP sequencer spends
    # 565ns per trigger, which was the feed bottleneck with small pieces).
    #   xT:   [128, c*KI+kt, ch]   piece (c) = 2KB/partition
    #   wihp: [mb*128+p, kt, col]  piece (mb) = 2KB/partition
    #   whhp: [mb*128+p, kt, col]  piece (mb) = 8KB/partition
    #   whop: [p, kt, col]         8KB/partition
    xT = nc.dram_tensor("xT", [128, NCH * KI, CH], FP8,
                        kind="ExternalInput").ap()
    wihp = nc.dram_tensor("wihp", [NB * 128, KI, 512], FP8,
                          kind="ExternalInput").ap()
    whhp = nc.dram_tensor("whhp", [NB * 128, KH, 512], FP8,
                          kind="ExternalInput").ap()
    whop = nc.dram_tensor("whop", [128, KH, NO], FP8,
                          kind="ExternalInput").ap()
    wio = nc.dram_tensor("wio", [128, KI, NO], FP8, kind="ExternalInput").ap()
    hbc = nc.dram_tensor("hbc", [128, KH], F32, kind="ExternalInput").ap()
    obc = nc.dram_tensor("obc", [128, KO], F32, kind="ExternalInput").ap()
    bmask = nc.dram_tensor("bmask", [128, 2 * CH], mybir.dt.uint8,
                           kind="ExternalInput").ap()
    outT = nc.dram_tensor("outT", [NO, BL], BF16, kind="ExternalOutput").ap()

    with tile.TileContext(nc) as tc:
        with tc.tile_pool(name="w", bufs=1) as wpool, \
             tc.tile_pool(name="act", bufs=1) as apool, \
             tc.tile_pool(name="ps", bufs=2, space="PSUM") as pspool, \
             tc.tile_pool(name="out", bufs=4) as opool:

            wih_t = [wpool.tile([128, KI, 512], FP8, tag=f"wih{mb}",
                                name=f"wih{mb}") for mb in range(NB)]
            whh_t = [wpool.tile([128, KH, 512], FP8, tag=f"whh{mb}",
                                name=f"whh{mb}") for mb in range(NB)]
            who_m = wpool.tile([128, KH, NO], FP8, tag="who", name="whom")
            x_m = wpool.tile([128, NCH * KI, CH], FP8, tag="x", name="xm")
            wio_m = wpool.tile([128, KI, NO], FP8, tag="wio", name="wiom")
            hbc_t = wpool.tile([128, KH], F32, tag="hbc")
            obc_t = wpool.tile([128, KO], F32, tag="obc")
            bmask_t = wpool.tile([128, 2 * CH], mybir.dt.uint8, tag="bmask")

            # ---- PE warmup: dummy matmuls gated only on a DVE memset (the
            # DVE sequencer comes alive ~1us before gpsimd), so the p-state
            # ramp and HAM clock-gate run against garbage work while the
            # first real operands are still in flight (~10.3us). 12 x 256
            # columns spans ~3us of PE time from a ~7.4us start. ----
            warm_t = wpool.tile([128, 2, 256], FP8, tag="warm", name="warm")
            nc.vector.memset(warm_t[:], 0.0)
            # dummy Sigmoid first so the greedy act-table pass picks the set
            # containing sigmoid+tanh+relu+copy — one table load in the DMA
            # shadow instead of a second 1.28us load mid-ACT-stream
            warm_o = wpool.tile([128, 4], BF16, tag="warmo", name="warmo")
            nc.scalar.activation(warm_o[:], warm_t[:, 0, 0:4], AF.Sigmoid)
            ps_w = pspool.tile([128, 4 * CH], F32, tag="ps", name="psw")
            for _w in range(18):
                nc.tensor.matmul(
                    ps_w[:, (_w % 4) * CH:(_w % 4) * CH + 256],
                    warm_t[:, :, 0:128], warm_t[:],
                    start=True, stop=True, perf_mode=DR,
                    skip_group_check=True)

            # ---- stage all inputs in exact consumption order ----
            # ALL large operands go on the single SP HWDGE queue: with 1-8KB
            # descriptors one queue saturates the 16 shared DMA engines
            # (~360GB/s measured), and strict FIFO makes arrival order equal
            # consumption order. (A parallel gpsimd software-DGE stream for
            # whh was tried and starved the x/wih stream it was supposed to
            # overlap with — the engines favor the software queue.)
            # The queue admits ~5 transfers in flight and the engines
            # round-robin across them, so a window of pieces completes
            # roughly together. The x-proj operands go first and fly solo;
            # the whh stream is held back by a 4-byte DVE "gate" copy that
            # reads the tail of the last x piece and writes a corner of
            # whh_t[0] — the whh trigger's write-after-write dependency on
            # that corner keeps the 1MB whh pieces out of the in-flight
            # window until the x stream has landed.
            MD = dict(max_dma_last_dim=2048)
            nc.sync.dma_start(x_m[:, 0:KI, :], xT[:, 0:KI, :], **MD)
            for mb in range(NB):
                nc.sync.dma_start(wih_t[mb][:],
                                  wihp[mb * 128:(mb + 1) * 128], **MD)
            nc.sync.dma_start(x_m[:, KI:2 * KI, :], xT[:, KI:2 * KI, :], **MD)
            # (the gate runs on gpsimd — it is idle, and at the head of the
            # DVE queue the wait for x-c1 would push the whole cast chain)
            nc.gpsimd.tensor_copy(whh_t[0][0:1, 0:1, 0:4],
                                  x_m[0:1, 2 * KI - 1:2 * KI, CH - 4:CH])
            for mb in range(NB):
                nc.sync.dma_start(whh_t[mb][:],
                                  whhp[mb * 128:(mb + 1) * 128], **MD)
            nc.sync.dma_start(who_m[:], whop[:], **MD)
            # ACT HWDGE queue: only the small early operands (more triggers
            # here would eat scalar-sequencer time the ACT backlog needs).
            nc.scalar.dma_start(hbc_t[:], hbc[:])
            nc.scalar.dma_start(bmask_t[:], bmask[:])
            nc.scalar.dma_start(wio_m[:], wio[:])
            nc.scalar.dma_start(obc_t[:], obc[:])

            # ---- per-chunk x-projection P and first-step activations.
            # P holds the raw WS-scaled x-projection (bias is applied inside
            # the ACTs); the copy out of PSUM frees the slot for the next
            # group, and the hh phase can't start until c1's last two slots
            # are freed. The DVE alone would need 18.4us for all eight
            # copies vs 13.8us of x-proj PE work, so c1's blk1-3 copy via
            # scalar ACT(Copy); emission is choreographed so those casts run
            # ahead of c0's blk3 ACTs and all of c1's ACTs in the scalar
            # queue — the A1 activations they displace have 25+us of
            # deadline slack, the PSUM slots don't. ----
            P = {}
            A = {}
            a1 = {}
            deferred = {0: [], 1: []}
            late_acts = {0: [], 1: []}
            for c in range(NCH):
                P[c] = apool.tile([128, KH * CH], BF16, tag=f"P{c}",
                                  name=f"P{c}")
                a1[c] = apool.tile([128, KH, CH], FP8, tag="A", bufs=3,
                                   name=f"A1c{c}")
                for blk in range(4):
                    ps = pspool.tile([128, 4 * CH], F32, tag="ps", name="psb")
                    for kp in range(KI // 2):
                        for mloc in range(4):
                            nc.tensor.matmul(
                                ps[:, mloc * CH:(mloc + 1) * CH],
                                wih_t[blk][:, 2 * kp:2 * kp + 2,
                                           mloc * 128:(mloc + 1) * 128],
                                x_m[:, c * KI + 2 * kp:c * KI + 2 * kp + 2, :],
                                start=(kp == 0), stop=(kp == KI // 2 - 1),
                                perf_mode=DR)
                    dst = P[c][:, blk * 4 * CH:(blk + 1) * 4 * CH]
                    if c == 1 and blk >= 1:
                        nc.scalar.activation(dst, ps[:], AF.Copy)
                    else:
                        nc.vector.tensor_copy(dst, ps[:])
                    if c == 0 and blk < 3:
                        _emit_hidden_act(nc, dst, blk, a1[c], opool, bmask_t,
                                         hbc_t, deferred[c])
                    else:
                        late_acts[c].append((dst, blk))
            for c in range(NCH):
                for dst, blk in late_acts[c]:
                    _emit_hidden_act(nc, dst, blk, a1[c], opool, bmask_t,
                                     hbc_t, deferred[c])
                _flush_deferred(nc, deferred[c])
                A[c] = a1[c]

            # ---- recurrent steps 2..4, one chunk at a time: chunk c's
            # next-step consumer is a full 27.6us behind its producer, so
            # the scalar ACT backlog (11us per chunk-step) never gates ----
            def hh_step(c, s):
                a_new = apool.tile([128, KH, CH], FP8, tag="A", bufs=3,
                                   name=f"A{s + 2}c{c}")
                deferred = []
                for blk in range(4):
                    ps = pspool.tile([128, 4 * CH], F32, tag="ps", name="psb")
                    for kp in range(KH // 2):
                        for mloc in range(4):
                            nc.tensor.matmul(
                                ps[:, mloc * CH:(mloc + 1) * CH],
                                whh_t[blk][:, 2 * kp:2 * kp + 2,
                                           mloc * 128:(mloc + 1) * 128],
                                A[c][:, 2 * kp:2 * kp + 2, :],
                                start=(kp == 0), stop=(kp == KH // 2 - 1),
                                perf_mode=DR)
                    # pre = psum + P into an SBUF temp: a single PSUM read
                    # frees the bank; ACT then runs off SBUF
                    tmp = opool.tile([128, 4 * CH], F32, tag="pre", bufs=2,
                                     name="pre")
                    nc.vector.tensor_add(
                        tmp[:], ps[:],
                        P[c][:, blk * 4 * CH:(blk + 1) * 4 * CH])
                    _emit_hidden_act(nc, tmp, blk, a_new, opool, bmask_t,
                                     hbc_t, deferred)
                _flush_deferred(nc, deferred)
                A[c] = a_new

            for s in range(N_STEPS - 1):
                for c in range(NCH):
                    hh_step(c, s)

            # ---- output layer; chunk 0's output post-chain overlaps chunk
            # 1's final hh step and output matmuls ----
            # ---- output layer: the x-projection (wio) matmuls open each
            # PSUM accumulation group and the who matmuls close it, so the
            # whole out pre-activation stays in f32 PSUM and the sigmoid ACT
            # reads PSUM directly — no SBUF outx tile, no DVE adds, and the
            # tail chain after the last matmul is just ACT + store ----
            def out_chunk(c):
                last = (c == NCH - 1)
                for mo in range(KO):
                    pso = pspool.tile([128, CH], F32, tag="ps", name="pso")
                    oap = pso[:]
                    for kp in range(KI // 2):
                        nc.tensor.matmul(
                            oap,
                            wio_m[:, 2 * kp:2 * kp + 2,
                                  mo * 128:(mo + 1) * 128],
                            x_m[:, c * KI + 2 * kp:c * KI + 2 * kp + 2, :],
                            start=(kp == 0), stop=False, perf_mode=DR)
                    for kp in range(KH // 2):
                        nc.tensor.matmul(
                            oap,
                            who_m[:, 2 * kp:2 * kp + 2,
                                  mo * 128:(mo + 1) * 128],
                            A[c][:, 2 * kp:2 * kp + 2, :],
                            start=False, stop=(kp == KH // 2 - 1),
                            perf_mode=DR)
                    o = opool.tile([128, CH], BF16, tag="o", bufs=2, name="o")
                    if last and mo == KO - 1:
                        # split the very last tile 384/128 so the ACT/store
                        # chain pipelines and the final exposed piece is
                        # small
                        for lo, hi in ((0, 384), (384, 512)):
                            hs = slice(lo, hi)
                            nc.scalar.activation(
                                o[:, hs], pso[:, hs], AF.Sigmoid,
                                bias=obc_t[:, mo:mo + 1], scale=IWS)
                            nc.sync.dma_start(
                                outT[mo * 128:(mo + 1) * 128,
                                     c * CH + lo:c * CH + hi],
                                o[:, hs])
                    else:
                        nc.scalar.activation(o[:], oap, AF.Sigmoid,
                                             bias=obc_t[:, mo:mo + 1],
                                             scale=IWS)
                        eng = nc.sync if mo % 2 == 0 else nc.scalar
                        eng.dma_start(
                            outT[mo * 128:(mo + 1) * 128,
                                 c * CH:(c + 1) * CH],
                            o[:])

            out_chunk(0)
            out_chunk(1)

    nc.compile()
    return nc


_NC_CACHE = None


def _get_nc():
    global _NC_CACHE
    if _NC_CACHE is None:
        _NC_CACHE = _build_nc()
    return _NC_CACHE


def _make_bmask():
    m = np.zeros((128, 2 * CH), np.uint8)
    m[:_B1 - (_B1 // 128) * 128, 0:CH] = 1          # tile 5: parts < 43 tanh
    m[_B2 - (_B2 // 128) * 128:, CH:2 * CH] = 1     # tile 10: parts >= 86 relu
    return m


def _q8(a):
    return np.clip(a, -240.0, 240.0).astype(E4)


def _prep_in_maps(inputs):
    x = np.asarray(inputs["inputs"], np.float32)
    hr = np.asarray(inputs["hidden_responses"], np.float32)[PERM]
    hb = np.asarray(inputs["hidden_biases"], np.float32)[PERM]
    orr = np.asarray(inputs["output_responses"], np.float32)
    ob = np.asarray(inputs["output_biases"], np.float32)

    wih_s = WS * (hr[:, None] * np.asarray(inputs["input_to_hidden"], np.float32)[PERM]).T
    whh_s = WS * (hr[:, None] *
                  np.asarray(inputs["hidden_to_hidden"], np.float32)[PERM][:, PERM]).T
    who_s = WS * (orr[:, None] *
                  np.asarray(inputs["hidden_to_output"], np.float32)[:, PERM]).T
    wio_s = WS * (orr[:, None] * np.asarray(inputs["input_to_output"], np.float32)).T

    def pack(w, ktiles):     # (ktiles*128, C) -> (128, ktiles, C)
        c = w.shape[1]
        return np.ascontiguousarray(
            w.reshape(ktiles, 128, c).transpose(1, 0, 2))

    def pack_mb(w, ktiles, mw, kw=None):
        # (ktiles*128, C) -> pieces [(mb, kq)][128, kw, mw] stacked on dim0:
        # piece (mb, kq) holds k-tiles kq*kw..(kq+1)*kw of columns
        # mb*mw..(mb+1)*mw, contiguous per partition on both sides
        kw = kw or ktiles
        p = pack(w, ktiles)                       # (128, ktiles, C)
        nmb, nkq = p.shape[2] // mw, ktiles // kw
        return np.ascontiguousarray(
            p.reshape(128, nkq, kw, nmb, mw).transpose(3, 1, 0, 2, 4)
            .reshape(nmb * nkq * 128, kw, mw))

    shared = {
        "wihp": _q8(pack_mb(wih_s, KI, 512)),
        "whhp": _q8(pack_mb(whh_s, KH, 512)),
        "whop": _q8(pack(who_s, KH)),
        "wio": _q8(pack(wio_s, KI)),
        "hbc": np.ascontiguousarray(hb.reshape(KH, 128).T),
        "obc": np.ascontiguousarray(ob.reshape(KO, 128).T),
        "bmask": _make_bmask(),
    }
    in_maps = []
    for c in range(N_CORES):
        m = dict(shared)
        xp = pack(np.ascontiguousarray(x[c * BL:(c + 1) * BL].T), KI)
        m["xT"] = _q8(np.ascontiguousarray(
            xp.reshape(128, KI, NCH, CH).transpose(0, 2, 1, 3)
            .reshape(128, NCH * KI, CH)))
        in_maps.append(m)
    return in_maps


def _run(inputs, trace=False, tmpdir=None):
    nc = _get_nc()
    in_maps = _prep_in_maps(inputs)
    res = run_bass_kernel_spmd(nc, in_maps, core_ids=list(range(N_CORES)),
                               trace=trace, tmpdir=tmpdir)
    out = np.empty((B, NO), np.float32)
    for c in range(N_CORES):
        out[c * BL:(c + 1) * BL] = res.results[c]["outT"].T.astype(np.float32)
    return out, res


def kernel(**inputs) -> np.ndarray:
    out, _ = _run(inputs, trace=False)
    return out


if __name__ == "__main__":
    rng = np.random.default_rng(0)
    ins = {
        "inputs": rng.standard_normal((B, NI), dtype=np.float32),
        "input_to_hidden": rng.standard_normal((NH, NI), dtype=np.float32) * 0.02,
        "hidden_to_hidden": rng.standard_normal((NH, NH), dtype=np.float32) * 0.02,
        "output_to_hidden": rng.standard_normal((NH, NO), dtype=np.float32) * 0.02,
        "input_to_output": rng.standard_normal((NO, NI), dtype=np.float32) * 0.02,
        "hidden_to_output": rng.standard_normal((NO, NH), dtype=np.float32) * 0.02,
        "output_to_output": rng.standard_normal((NO, NO), dtype=np.float32) * 0.02,
        "hidden_responses": rng.standard_normal(NH, dtype=np.float32) * 0.1 + 1.0,
        "hidden_biases": rng.standard_normal(NH, dtype=np.float32) * 0.1,
        "output_responses": rng.standard_normal(NO, dtype=np.float32) * 0.1 + 1.0,
        "output_biases": rng.standard_normal(NO, dtype=np.float32) * 0.1,
    }
    out = kernel(**ins)
    print("kernel output", out.shape, out.dtype, out[:2, :4])
